# revision 26
# baseline (speedup 1.0000x reference)
"""Multi-headed self-attention (B=2, S=2048, D=1024, H=16) on 8 TRN2 cores.

Sharding: hybrid batch x head tensor-parallel. Core c handles batch c//4 and
heads (c%4)*4 .. (c%4)*4+3 (two head-pairs). Host sums the 4 partials per
batch.

Key design points (vs the f32r feature-major baseline):
- x = query + pos_emb is computed on host (fp32) and shipped transposed, so
  pos_emb is never transferred and no device add is needed.
- The QKV projection contracts over K=128 partitions in f32r (full rate).
  q/k/P/V are stored fp16: with K=64 contraction the PE streams 2-byte moving
  data at 1 row/cycle where f32r drops to ~half rate, and fp16 keeps 11
  mantissa bits (~4x bf16 precision). exp uses bias=-4 so P fits fp16 range.
- AV runs token-major: P is the stationary operand (ldweights overlap with
  streaming, ~30-55ns per 65-row matmul) and V a 65-column moving operand
  whose last column is ones (set by memset, not a strided DMA), so softmax
  denominators land in psum column 64 and normalization is a per-partition
  reciprocal + scale on the vector engine.
- Attention is 8 jobs = (head, query-half); each job's AV is emitted one job
  behind its QK/exp so the scalar engine (exp, the pacing engine) never idles
  at job boundaries.
- AV output is token-major [q, dims]; a fp16 PE transpose per 128x128 block
  restores feature-major oT for the output projection, reusing the AV psum
  pool.
"""

import os
import sys

import numpy as np

if "/opt/trn_rl_repo" not in sys.path:
    sys.path.insert(0, "/opt/trn_rl_repo")

B, S, D, H = 2, 2048, 1024, 16
DK = 64
P = 128
NCORES = 8
HPC = H // (NCORES // B)  # heads per core = 4
T = S  # tokens per core (one batch)
NDC = D // P  # 8 contraction chunks
NTB = T // P  # 16 token blocks
SCALE = DK**-0.5
EXP_BIAS = -4.0

_CACHE = {}


def _build_program(reps=1):
    from contextlib import ExitStack, nullcontext

    import concourse.bass as bass
    import concourse.tile as tile
    from concourse import bacc
    from concourse import mybir
    from concourse.masks import make_identity

    f32 = mybir.dt.float32
    f32r = mybir.dt.float32r
    fp16 = mybir.dt.float16
    EXP = mybir.ActivationFunctionType.Exp

    nc = bacc.Bacc()
    xT = nc.declare_dram_parameter("xT", [D, T], fp16, isOutput=False)
    wqk = nc.declare_dram_parameter("wqk", [D, 4 * P], fp16, isOutput=False)
    wv = nc.declare_dram_parameter("wv", [D, 2 * P], fp16, isOutput=False)
    wout = nc.declare_dram_parameter("wout", [2 * P, D], fp16, isOutput=False)
    out = nc.declare_dram_parameter("out", [T, D], f32, isOutput=True)

    with tile.TileContext(nc) as tc, ExitStack() as top:
        const = top.enter_context(tc.tile_pool(name="const", bufs=1))
        wout_sb = const.tile([P, 2, D], fp16)  # 4KB/p
        qkT = const.tile([P, 4, T], fp16)  # q0,k0,q1,k1 feature-major, 16KB/p
        V_sb = const.tile([P, NTB, HPC, DK + 1], fp16)  # token-major V, 8.3KB/p
        o_sb = const.tile([P, 2, NTB, P], fp16)  # normalized o token-major, 8KB/p
        oT = const.tile([P, 2, T], fp16)  # feature-major o, 8KB/p
        vT = const.tile([P, 2, T], fp16)  # feature-major v staging, 8KB/p
        identh = const.tile([P, P], fp16)
        make_identity(nc, identh[:])
        biasc = const.tile([P, 1], f32)
        nc.gpsimd.memset(biasc[:], EXP_BIAS)

        rep_ctx = tc.For_i(0, reps, 1) if reps > 1 else nullcontext()
        top.enter_context(rep_ctx)

        # ones column for the AV denominator trick
        for h in range(HPC):
            nc.gpsimd.memset(V_sb[:, :, h, DK : DK + 1], 1.0)

        # ---- unified pipeline ----
        # ec0/ec1 (pair-0 q,k) projection runs up front; everything else
        # (v projection, pair-1 q,k projection, AV, normalize, transposes,
        # output projection) is chunked into a pending queue drained one
        # chunk per kb inside the QK/exp loops, so the PE fills the slack
        # while the scalar engine streams exp.
        x_sb = const.tile([P, NDC, T], fp16)  # 32KB/p
        wqk_sb = const.tile([P, NDC, 4 * P], fp16)  # 8KB/p
        wv_sb = const.tile([P, NDC, 2 * P], fp16)  # 4KB/p
        for dc in range(NDC):
            nc.sync.dma_start(x_sb[:, dc, :], xT[dc * P : (dc + 1) * P, :])
            nc.sync.dma_start(wqk_sb[:, dc, :], wqk[dc * P : (dc + 1) * P, :])
        for dc in range(NDC):
            nc.sync.dma_start(wv_sb[:, dc, :], wv[dc * P : (dc + 1) * P, :])
        nc.sync.dma_start(wout_sb[:], wout.rearrange("(c p) n -> p c n", p=P))

        with (
            tc.tile_pool(name="pqk", bufs=3, space="PSUM") as pqk_pool,
            tc.tile_pool(name="pav", bufs=5, space="PSUM") as pav_pool,
            tc.tile_pool(name="pp", bufs=3) as p_pool,
            tc.tile_pool(name="rc", bufs=4) as rec_pool,
            tc.tile_pool(name="osb", bufs=3) as osb_pool,
        ):
            from collections import deque

            pending = deque()

            def slot():
                return pav_pool.tile([P, 512], f32, name="sl", tag="av")

            def qkproj_chunk(ec, tq):
                # qkT[:, ec, tq*512:(tq+1)*512] = wqk_chunk^T @ x
                ps = slot()
                for dc in range(NDC):
                    nc.tensor.matmul(
                        ps[:],
                        wqk_sb[:, dc, ec * P : (ec + 1) * P],
                        x_sb[:, dc, tq * 512 : (tq + 1) * 512],
                        start=(dc == 0),
                        stop=(dc == NDC - 1),
                    )
                nc.vector.tensor_copy(qkT[:, ec, tq * 512 : (tq + 1) * 512], ps[:])

            def proj_pieces(w_sb, wc, dst, dcol, tq):
                # an 8-deep projection chain split into two 4-dc pieces that
                # share one psum slot, so each drained piece steals less PE
                # time from the QK/exp stream
                st = {}
                def go1():
                    st["ps"] = slot()
                    for dc in range(NDC // 2):
                        nc.tensor.matmul(
                            st["ps"][:],
                            w_sb[:, dc, wc * P : (wc + 1) * P],
                            x_sb[:, dc, tq * 512 : (tq + 1) * 512],
                            start=(dc == 0),
                            stop=False,
                        )
                def go2():
                    ps = st["ps"]
                    for dc in range(NDC // 2, NDC):
                        nc.tensor.matmul(
                            ps[:],
                            w_sb[:, dc, wc * P : (wc + 1) * P],
                            x_sb[:, dc, tq * 512 : (tq + 1) * 512],
                            start=False,
                            stop=(dc == NDC - 1),
                        )
                    nc.vector.tensor_copy(dst[:, dcol, tq * 512 : (tq + 1) * 512], ps[:])
                return go1, go2

            def vtr_chunk(vc, tb0):
                def go():
                    # transpose vT 128-blocks into token-major V_sb
                    for tb in range(tb0, tb0 + 4):
                        tr = slot()
                        trh = tr.bitcast(fp16)[:, 0:P]
                        nc.tensor.transpose(
                            trh, vT[:, vc, tb * P : (tb + 1) * P], identh[:]
                        )
                        nc.vector.tensor_copy(
                            V_sb[:, tb, 2 * vc : 2 * vc + 2, 0:DK],
                            trh.rearrange("p (h d) -> p h d", h=2),
                        )
                return go

            def av_chunk(h, half, Pt, qb0):
                # two query blocks per chunk, matmuls interleaved across two
                # psum slots to amortize chunk-boundary sync
                def go():
                    pair, row = h // 2, (h % 2) * DK
                    avs = [slot(), slot()]
                    for kb in range(NTB):
                        for j in range(2):
                            nc.tensor.matmul(
                                avs[j][:, 0 : DK + 1],
                                Pt[:, kb, (qb0 + j) * P : (qb0 + j + 1) * P],
                                V_sb[:, kb, h, :],
                                start=(kb == 0),
                                stop=(kb == NTB - 1),
                            )
                    for j in range(2):
                        qg = half * 8 + qb0 + j
                        rec = rec_pool.tile([P, 1], f32, name="rec", tag="rec")
                        nc.vector.reciprocal(rec[:], avs[j][:, DK : DK + 1])
                        nc.vector.tensor_scalar_mul(
                            o_sb[:, pair, qg, row : row + DK],
                            avs[j][:, 0:DK],
                            rec[:],
                        )
                return go

            def tr_chunk(pair, qb0):
                def go():
                    for qb in range(qb0, qb0 + 4):
                        tr = slot()
                        trh = tr.bitcast(fp16)[:, 0:P]
                        nc.tensor.transpose(trh, o_sb[:, pair, qb, :], identh[:])
                        nc.vector.tensor_copy(
                            oT[:, pair, qb * P : (qb + 1) * P], trh
                        )
                return go

            def oproj_chunk(tb):
                # nh0/nh1 chains interleaved so pair-accumulation RAW latency
                # overlaps the other chain's stream
                def go():
                    ob = osb_pool.tile([P, D], f32, name="ob", tag="ob")
                    pos = [slot(), slot()]
                    for pair in range(2):
                        for nh in range(2):
                            nc.tensor.matmul(
                                pos[nh][:],
                                oT[:, pair, tb * P : (tb + 1) * P],
                                wout_sb[:, pair, nh * 512 : (nh + 1) * 512],
                                start=(pair == 0),
                                stop=(pair == 1),
                            )
                    for nh in range(2):
                        nc.vector.tensor_copy(
                            ob[:, nh * 512 : (nh + 1) * 512], pos[nh][:]
                        )
                    nc.sync.dma_start(out[tb * P : (tb + 1) * P, :], ob[:])
                return go

            def emit_qk_exp(h, half, drain=1):
                pair, row = h // 2, (h % 2) * DK
                q0 = half * 1024
                Pt = p_pool.tile([P, NTB, 1024], fp16, name="pt", tag="pt")
                for kb in range(NTB):
                    for hh in range(2):
                        pq = pqk_pool.tile([P, 512], f32, name="pq", tag="pq")
                        nc.tensor.matmul(
                            pq[:],
                            qkT[row : row + DK, 2 * pair + 1, kb * P : (kb + 1) * P],
                            qkT[
                                row : row + DK,
                                2 * pair,
                                q0 + hh * 512 : q0 + (hh + 1) * 512,
                            ],
                            start=True,
                            stop=True,
                        )
                        nc.scalar.activation(
                            Pt[:, kb, hh * 512 : (hh + 1) * 512],
                            pq[:],
                            EXP,
                            bias=biasc[:],
                            scale=SCALE,
                        )
                    for _ in range(drain):
                        if pending:
                            pending.popleft()()
                return Pt

            # upfront: pair-0 q,k projection, dc-outer with concurrent psum
            # chains so the matmuls chase the arriving x DMA chunks instead of
            # serializing after the full load
            for specs in [[(1, 0), (1, 1), (0, 0), (0, 1)], [(1, 2), (1, 3)]]:
                ups = [slot() for _ in specs]
                for dc in range(NDC):
                    for i, (ec, tq) in enumerate(specs):
                        nc.tensor.matmul(
                            ups[i][:],
                            wqk_sb[:, dc, ec * P : (ec + 1) * P],
                            x_sb[:, dc, tq * 512 : (tq + 1) * 512],
                            start=(dc == 0),
                            stop=(dc == NDC - 1),
                        )
                for i, (ec, tq) in enumerate(specs):
                    nc.vector.tensor_copy(
                        qkT[:, ec, tq * 512 : (tq + 1) * 512], ups[i][:]
                    )
            for tq in range(2, 4):
                pending.extend(proj_pieces(wqk_sb, 0, qkT, 0, tq))
            # deferred: v projection (feature-major + transpose), then pair-1 q,k
            for vc in range(2):
                for tq in range(4):
                    pending.extend(proj_pieces(wv_sb, vc, vT, vc, tq))
            for vc in range(2):
                for tb0 in range(0, NTB, 4):
                    pending.append(vtr_chunk(vc, tb0))
            for ec in range(2, 4):
                for tq in range(4):
                    pending.extend(proj_pieces(wqk_sb, ec, qkT, ec, tq))

            def enqueue_post_av(ph, phalf):
                # after the pair's second head finishes a half, its transposes
                # can run; after pair 1's transposes, those token blocks can be
                # output-projected (both pairs' oT ready).
                if ph % 2 == 1:
                    pair, qg0 = ph // 2, phalf * 8
                    pending.append(tr_chunk(pair, qg0))
                    pending.append(tr_chunk(pair, qg0 + 4))
                    if pair == 1:
                        for tb in range(qg0, qg0 + 8, 2):
                            def op(tb=tb):
                                def go():
                                    oproj_chunk(tb)()
                                    oproj_chunk(tb + 1)()
                                return go
                            pending.append(op())

            jobs = [(h, half) for h in range(HPC) for half in range(2)]
            prev = None
            for ji, (h, half) in enumerate(jobs):
                Pt = emit_qk_exp(h, half, drain=2 if ji >= len(jobs) - 2 else 1)
                if prev is not None:
                    ph, phalf, pPt = prev
                    for qb0 in range(0, 8, 2):
                        pending.append(av_chunk(ph, phalf, pPt, qb0))
                    enqueue_post_av(ph, phalf)
                prev = (h, half, Pt)
            ph, phalf, pPt = prev
            for qb0 in range(0, 8, 2):
                pending.append(av_chunk(ph, phalf, pPt, qb0))
            enqueue_post_av(ph, phalf)
            while pending:
                pending.popleft()()

    nc.compile()
    return nc


def get_program():
    if "nc" not in _CACHE:
        _CACHE["nc"] = _build_program()
    return _CACHE["nc"]


def make_in_maps(query, pos_emb, w_qkv, w_out):
    query = np.asarray(query, dtype=np.float32)
    pos_emb = np.asarray(pos_emb, dtype=np.float32)
    w_qkv = np.asarray(w_qkv, dtype=np.float32)
    w_out = np.asarray(w_out, dtype=np.float32)
    xTs = [np.ascontiguousarray((query[b] + pos_emb).T.astype(np.float16)) for b in range(B)]
    in_maps = []
    for c in range(NCORES):
        b, hb = c // (NCORES // B), (c % (NCORES // B)) * HPC
        heads = list(range(hb, hb + HPC))
        # w_qkv column e for head h, kind j (q/k/v), dim d: e = h*3*DK + j*DK + d
        qk_cols = []
        for pair in range(2):
            for j in range(2):  # q then k
                for h in heads[2 * pair : 2 * pair + 2]:
                    base = h * 3 * DK + j * DK
                    qk_cols.append(w_qkv[:, base : base + DK])
        wqk_c = np.ascontiguousarray(np.concatenate(qk_cols, axis=1).astype(np.float16))
        wv_c = np.ascontiguousarray(
            np.concatenate(
                [w_qkv[:, h * 3 * DK + 2 * DK : h * 3 * DK + 3 * DK] for h in heads],
                axis=1,
            ).astype(np.float16)
        )
        wout_c = np.concatenate(
            [w_out[h * DK : (h + 1) * DK, :] for h in heads], axis=0
        ).astype(np.float16)
        in_maps.append(
            {"xT": xTs[b], "wqk": wqk_c, "wv": wv_c, "wout": wout_c}
        )
    return in_maps


def gather_output(results):
    out = np.zeros((B, S, D), dtype=np.float32)
    for c in range(NCORES):
        out[c // (NCORES // B)] += results[c]["out"]
    return out


def kernel(query, pos_emb, w_qkv, w_out):
    from concourse.bass_utils import run_bass_kernel_spmd

    nc = get_program()
    in_maps = make_in_maps(query, pos_emb, w_qkv, w_out)
    res = run_bass_kernel_spmd(nc, in_maps, list(range(NCORES)))
    return gather_output(res.results)


# revision 27
# speedup vs baseline: 3.5231x; 3.5231x over previous
"""Multi-headed self-attention (B=2, S=2048, D=1024, H=16) on 8 TRN2 cores.

Sharding: hybrid batch x head tensor-parallel. Core c handles batch c//4 and
heads (c%4)*4 .. (c%4)*4+3 (two head-pairs). Host sums the 4 partials per
batch.

Key design points (vs the f32r feature-major baseline):
- x = query + pos_emb is computed on host (fp32) and shipped transposed, so
  pos_emb is never transferred and no device add is needed.
- The QKV projection contracts over K=128 partitions in f32r (full rate).
  q/k/P/V are stored fp16: with K=64 contraction the PE streams 2-byte moving
  data at 1 row/cycle where f32r drops to ~half rate, and fp16 keeps 11
  mantissa bits (~4x bf16 precision). exp uses bias=-4 so P fits fp16 range.
- AV runs token-major: P is the stationary operand (ldweights overlap with
  streaming, ~30-55ns per 65-row matmul) and V a 65-column moving operand
  whose last column is ones (set by memset, not a strided DMA), so softmax
  denominators land in psum column 64 and normalization is a per-partition
  reciprocal + scale on the vector engine.
- Attention is 8 jobs = (head, query-half); each job's AV is emitted one job
  behind its QK/exp so the scalar engine (exp, the pacing engine) never idles
  at job boundaries.
- AV output is token-major [q, dims]; a fp16 PE transpose per 128x128 block
  restores feature-major oT for the output projection, reusing the AV psum
  pool.
"""

import os
import sys

import numpy as np

if "/opt/trn_rl_repo" not in sys.path:
    sys.path.insert(0, "/opt/trn_rl_repo")

B, S, D, H = 2, 2048, 1024, 16
DK = 64
P = 128
NCORES = 8
HPC = H // (NCORES // B)  # heads per core = 4
T = S  # tokens per core (one batch)
NDC = D // P  # 8 contraction chunks
NTB = T // P  # 16 token blocks
SCALE = DK**-0.5
EXP_BIAS = -4.0

_CACHE = {}


def _build_program(reps=1):
    from contextlib import ExitStack, nullcontext

    import concourse.bass as bass
    import concourse.tile as tile
    from concourse import bacc
    from concourse import mybir
    from concourse.masks import make_identity

    f32 = mybir.dt.float32
    f32r = mybir.dt.float32r
    fp16 = mybir.dt.float16
    EXP = mybir.ActivationFunctionType.Exp

    nc = bacc.Bacc()
    xT = nc.declare_dram_parameter("xT", [D, T], fp16, isOutput=False)
    wqk = nc.declare_dram_parameter("wqk", [D, 4 * P], fp16, isOutput=False)
    wv = nc.declare_dram_parameter("wv", [D, 2 * P], fp16, isOutput=False)
    wout = nc.declare_dram_parameter("wout", [2 * P, D], fp16, isOutput=False)
    out = nc.declare_dram_parameter("out", [T, D], f32, isOutput=True)

    with tile.TileContext(nc) as tc, ExitStack() as top:
        const = top.enter_context(tc.tile_pool(name="const", bufs=1))
        wout_sb = const.tile([P, 2, D], fp16)  # 4KB/p
        qkT = const.tile([P, 4, T], fp16)  # q0,k0,q1,k1 feature-major, 16KB/p
        V_sb = const.tile([P, NTB, HPC, DK + 1], fp16)  # token-major V, 8.3KB/p
        o_sb = const.tile([P, 2, NTB, P], fp16)  # normalized o token-major, 8KB/p
        oT = const.tile([P, 2, T], fp16)  # feature-major o, 8KB/p
        vT = const.tile([P, 2, T], fp16)  # feature-major v staging, 8KB/p
        identh = const.tile([P, P], fp16)
        make_identity(nc, identh[:])
        biasc = const.tile([P, 1], f32)
        nc.gpsimd.memset(biasc[:], EXP_BIAS)

        rep_ctx = tc.For_i(0, reps, 1) if reps > 1 else nullcontext()
        top.enter_context(rep_ctx)

        # ones column for the AV denominator trick
        for h in range(HPC):
            nc.gpsimd.memset(V_sb[:, :, h, DK : DK + 1], 1.0)

        # ---- unified pipeline ----
        # ec0/ec1 (pair-0 q,k) projection runs up front; everything else
        # (v projection, pair-1 q,k projection, AV, normalize, transposes,
        # output projection) is chunked into a pending queue drained one
        # chunk per kb inside the QK/exp loops, so the PE fills the slack
        # while the scalar engine streams exp.
        x_sb = const.tile([P, NDC, T], fp16)  # 32KB/p
        wqk_sb = const.tile([P, NDC, 4 * P], fp16)  # 8KB/p
        wv_sb = const.tile([P, NDC, 2 * P], fp16)  # 4KB/p
        for dc in range(NDC):
            nc.sync.dma_start(x_sb[:, dc, :], xT[dc * P : (dc + 1) * P, :])
            nc.sync.dma_start(wqk_sb[:, dc, :], wqk[dc * P : (dc + 1) * P, :])
        for dc in range(NDC):
            nc.sync.dma_start(wv_sb[:, dc, :], wv[dc * P : (dc + 1) * P, :])
        nc.sync.dma_start(wout_sb[:], wout.rearrange("(c p) n -> p c n", p=P))

        with (
            tc.tile_pool(name="pqk", bufs=4, space="PSUM") as pqk_pool,
            tc.tile_pool(name="pav", bufs=4, space="PSUM") as pav_pool,
            tc.tile_pool(name="pp", bufs=3) as p_pool,
            tc.tile_pool(name="rc", bufs=4) as rec_pool,
            tc.tile_pool(name="osb", bufs=3) as osb_pool,
        ):
            from collections import deque

            pending = deque()

            def slot():
                return pav_pool.tile([P, 512], f32, name="sl", tag="av")

            def qkproj_chunk(ec, tq):
                # qkT[:, ec, tq*512:(tq+1)*512] = wqk_chunk^T @ x
                ps = slot()
                for dc in range(NDC):
                    nc.tensor.matmul(
                        ps[:],
                        wqk_sb[:, dc, ec * P : (ec + 1) * P],
                        x_sb[:, dc, tq * 512 : (tq + 1) * 512],
                        start=(dc == 0),
                        stop=(dc == NDC - 1),
                    )
                nc.vector.tensor_copy(qkT[:, ec, tq * 512 : (tq + 1) * 512], ps[:])

            def proj_pieces(w_sb, wc, dst, dcol, tq):
                # an 8-deep projection chain split into two 4-dc pieces that
                # share one psum slot, so each drained piece steals less PE
                # time from the QK/exp stream
                st = {}
                def go1():
                    st["ps"] = slot()
                    for dc in range(NDC // 2):
                        nc.tensor.matmul(
                            st["ps"][:],
                            w_sb[:, dc, wc * P : (wc + 1) * P],
                            x_sb[:, dc, tq * 512 : (tq + 1) * 512],
                            start=(dc == 0),
                            stop=False,
                        )
                def go2():
                    ps = st["ps"]
                    for dc in range(NDC // 2, NDC):
                        nc.tensor.matmul(
                            ps[:],
                            w_sb[:, dc, wc * P : (wc + 1) * P],
                            x_sb[:, dc, tq * 512 : (tq + 1) * 512],
                            start=False,
                            stop=(dc == NDC - 1),
                        )
                    nc.vector.tensor_copy(dst[:, dcol, tq * 512 : (tq + 1) * 512], ps[:])
                return go1, go2

            def vtr_chunk(vc, tb0):
                def go():
                    # transpose vT 128-blocks into token-major V_sb
                    for tb in range(tb0, tb0 + 4):
                        tr = slot()
                        trh = tr.bitcast(fp16)[:, 0:P]
                        nc.tensor.transpose(
                            trh, vT[:, vc, tb * P : (tb + 1) * P], identh[:]
                        )
                        nc.vector.tensor_copy(
                            V_sb[:, tb, 2 * vc : 2 * vc + 2, 0:DK],
                            trh.rearrange("p (h d) -> p h d", h=2),
                        )
                return go

            def av_chunk(h, half, Pt, qb0):
                # two query blocks per chunk, matmuls interleaved across two
                # psum slots to amortize chunk-boundary sync
                def go():
                    pair, row = h // 2, (h % 2) * DK
                    avs = [slot(), slot()]
                    for kb in range(NTB):
                        for j in range(2):
                            nc.tensor.matmul(
                                avs[j][:, 0 : DK + 1],
                                Pt[:, kb, (qb0 + j) * P : (qb0 + j + 1) * P],
                                V_sb[:, kb, h, :],
                                start=(kb == 0),
                                stop=(kb == NTB - 1),
                            )
                    for j in range(2):
                        qg = half * 8 + qb0 + j
                        rec = rec_pool.tile([P, 1], f32, name="rec", tag="rec")
                        nc.vector.reciprocal(rec[:], avs[j][:, DK : DK + 1])
                        nc.vector.tensor_scalar_mul(
                            o_sb[:, pair, qg, row : row + DK],
                            avs[j][:, 0:DK],
                            rec[:],
                        )
                return go

            def tr_chunk(pair, qb0):
                def go():
                    for qb in range(qb0, qb0 + 4):
                        tr = slot()
                        trh = tr.bitcast(fp16)[:, 0:P]
                        nc.tensor.transpose(trh, o_sb[:, pair, qb, :], identh[:])
                        nc.vector.tensor_copy(
                            oT[:, pair, qb * P : (qb + 1) * P], trh
                        )
                return go

            def oproj_chunk(tb):
                # nh0/nh1 chains interleaved so pair-accumulation RAW latency
                # overlaps the other chain's stream
                def go():
                    ob = osb_pool.tile([P, D], f32, name="ob", tag="ob")
                    pos = [slot(), slot()]
                    for pair in range(2):
                        for nh in range(2):
                            nc.tensor.matmul(
                                pos[nh][:],
                                oT[:, pair, tb * P : (tb + 1) * P],
                                wout_sb[:, pair, nh * 512 : (nh + 1) * 512],
                                start=(pair == 0),
                                stop=(pair == 1),
                            )
                    for nh in range(2):
                        nc.vector.tensor_copy(
                            ob[:, nh * 512 : (nh + 1) * 512], pos[nh][:]
                        )
                    nc.sync.dma_start(out[tb * P : (tb + 1) * P, :], ob[:])
                return go

            def emit_qk_exp(h, half, drain=1):
                pair, row = h // 2, (h % 2) * DK
                q0 = half * 1024
                Pt = p_pool.tile([P, NTB, 1024], fp16, name="pt", tag="pt")
                for kb in range(NTB):
                    for hh in range(2):
                        pq = pqk_pool.tile([P, 512], f32, name="pq", tag="pq")
                        nc.tensor.matmul(
                            pq[:],
                            qkT[row : row + DK, 2 * pair + 1, kb * P : (kb + 1) * P],
                            qkT[
                                row : row + DK,
                                2 * pair,
                                q0 + hh * 512 : q0 + (hh + 1) * 512,
                            ],
                            start=True,
                            stop=True,
                        )
                        nc.scalar.activation(
                            Pt[:, kb, hh * 512 : (hh + 1) * 512],
                            pq[:],
                            EXP,
                            bias=biasc[:],
                            scale=SCALE,
                        )
                    for _ in range(drain):
                        if pending:
                            pending.popleft()()
                return Pt

            # upfront: pair-0 q,k projection, dc-outer with concurrent psum
            # chains so the matmuls chase the arriving x DMA chunks instead of
            # serializing after the full load
            for specs in [[(1, 0), (1, 1), (0, 0), (0, 1)], [(1, 2), (1, 3)]]:
                ups = [slot() for _ in specs]
                for dc in range(NDC):
                    for i, (ec, tq) in enumerate(specs):
                        nc.tensor.matmul(
                            ups[i][:],
                            wqk_sb[:, dc, ec * P : (ec + 1) * P],
                            x_sb[:, dc, tq * 512 : (tq + 1) * 512],
                            start=(dc == 0),
                            stop=(dc == NDC - 1),
                        )
                for i, (ec, tq) in enumerate(specs):
                    nc.vector.tensor_copy(
                        qkT[:, ec, tq * 512 : (tq + 1) * 512], ups[i][:]
                    )
            for tq in range(2, 4):
                pending.extend(proj_pieces(wqk_sb, 0, qkT, 0, tq))
            # deferred: v projection (feature-major + transpose), then pair-1 q,k
            for vc in range(2):
                for tq in range(4):
                    pending.extend(proj_pieces(wv_sb, vc, vT, vc, tq))
            for vc in range(2):
                for tb0 in range(0, NTB, 4):
                    pending.append(vtr_chunk(vc, tb0))
            for ec in range(2, 4):
                for tq in range(4):
                    pending.extend(proj_pieces(wqk_sb, ec, qkT, ec, tq))

            def enqueue_post_av(ph, phalf):
                # after the pair's second head finishes a half, its transposes
                # can run; after pair 1's transposes, those token blocks can be
                # output-projected (both pairs' oT ready).
                if ph % 2 == 1:
                    pair, qg0 = ph // 2, phalf * 8
                    pending.append(tr_chunk(pair, qg0))
                    pending.append(tr_chunk(pair, qg0 + 4))
                    if pair == 1:
                        for tb in range(qg0, qg0 + 8, 2):
                            def op(tb=tb):
                                def go():
                                    oproj_chunk(tb)()
                                    oproj_chunk(tb + 1)()
                                return go
                            pending.append(op())

            jobs = [(h, half) for h in range(HPC) for half in range(2)]
            prev = None
            for ji, (h, half) in enumerate(jobs):
                Pt = emit_qk_exp(h, half, drain=2 if ji >= len(jobs) - 2 else 1)
                if prev is not None:
                    ph, phalf, pPt = prev
                    for qb0 in range(0, 8, 2):
                        pending.append(av_chunk(ph, phalf, pPt, qb0))
                    enqueue_post_av(ph, phalf)
                prev = (h, half, Pt)
            ph, phalf, pPt = prev
            for qb0 in range(0, 8, 2):
                pending.append(av_chunk(ph, phalf, pPt, qb0))
            enqueue_post_av(ph, phalf)
            while pending:
                pending.popleft()()

    nc.compile()
    return nc


def get_program():
    if "nc" not in _CACHE:
        _CACHE["nc"] = _build_program()
    return _CACHE["nc"]


def make_in_maps(query, pos_emb, w_qkv, w_out):
    query = np.asarray(query, dtype=np.float32)
    pos_emb = np.asarray(pos_emb, dtype=np.float32)
    w_qkv = np.asarray(w_qkv, dtype=np.float32)
    w_out = np.asarray(w_out, dtype=np.float32)
    xTs = [np.ascontiguousarray((query[b] + pos_emb).T.astype(np.float16)) for b in range(B)]
    in_maps = []
    for c in range(NCORES):
        b, hb = c // (NCORES // B), (c % (NCORES // B)) * HPC
        heads = list(range(hb, hb + HPC))
        # w_qkv column e for head h, kind j (q/k/v), dim d: e = h*3*DK + j*DK + d
        qk_cols = []
        for pair in range(2):
            for j in range(2):  # q then k
                for h in heads[2 * pair : 2 * pair + 2]:
                    base = h * 3 * DK + j * DK
                    qk_cols.append(w_qkv[:, base : base + DK])
        wqk_c = np.ascontiguousarray(np.concatenate(qk_cols, axis=1).astype(np.float16))
        wv_c = np.ascontiguousarray(
            np.concatenate(
                [w_qkv[:, h * 3 * DK + 2 * DK : h * 3 * DK + 3 * DK] for h in heads],
                axis=1,
            ).astype(np.float16)
        )
        wout_c = np.concatenate(
            [w_out[h * DK : (h + 1) * DK, :] for h in heads], axis=0
        ).astype(np.float16)
        in_maps.append(
            {"xT": xTs[b], "wqk": wqk_c, "wv": wv_c, "wout": wout_c}
        )
    return in_maps


def gather_output(results):
    out = np.zeros((B, S, D), dtype=np.float32)
    for c in range(NCORES):
        out[c // (NCORES // B)] += results[c]["out"]
    return out


def kernel(query, pos_emb, w_qkv, w_out):
    from concourse.bass_utils import run_bass_kernel_spmd

    nc = get_program()
    in_maps = make_in_maps(query, pos_emb, w_qkv, w_out)
    res = run_bass_kernel_spmd(nc, in_maps, list(range(NCORES)))
    return gather_output(res.results)
